# revision 24
# baseline (speedup 1.0000x reference)
"""CapsuleLayer (dynamic routing, 3 iterations) on 8 Trainium2 NeuronCores.

Math (see reference):
    x_hat[b,o,i,d] = sum_m W[o,i,d,m] * x[b,i,m]
    b_log = 0; for it in 0..2:
        c = softmax(b_log, axis=o)
        s = sum_i c[b,o,i] * x_hat[b,o,i,d]; out = squash(s)
        if it < 2: b_log += x_hat . out

Sharding: split I=512 over 8 cores (64 i's each).  The weight (the only
large operand, 134MB fp16) is then read exactly once from HBM.  The softmax
over o is local; only s[b,o,d] needs a cross-core AllReduce (3 sums total:
the first two on-device, the last one on the host as part of unsharding).

Per-core layout: x_hat resident in SBUF as fp16, 32 "i-pair" tiles
[128=(i2,b), (d,o)=2048] (d-major, o-minor: the big tensor_tensor
multiplies broadcast per-(o) vectors over d with a packed unit-stride
innermost o, which qualifies them for the DVE 2x 16-bit perf mode).

Key structure vs a naive mapping:
  - production matmuls use two concurrent 64x64 PE quadrants
    (tile_position) per i-pair; PSUM eviction tiles are [128,512] with 4
    buffers so the fp32->fp16 eviction copies run on DVE and ACT in
    parallel without pacing the PE.
  - the agreement logits use the identity  x_hat.out = f * (x_hat.s)
    where out = squash(s) = f(|s|)*s, so the dot products run against the
    raw AllReduced s (sg) and the scalar f is applied after the
    d-reduction on the small [128, j*o] tile.  This removes the full-size
    squash multiply and lets each d-half of the dot products start as
    soon as its AllReduce chunk lands.
  - s sums over i run on the PE with a block-diagonal [eye;eye]
    stationary; weighted sums (iters 1/2) are DVE multiplies (2x mode)
    chased by the PE, d-half by d-half, with the AllReduce of half 0
    overlapped against the compute of half 1.
  - all activation functions are from one ACT table set (exp/ln);
    sqrt(n2) is computed as exp(0.5*ln(n2)) to avoid ~2.7us table
    reloads between squash and softmax.
"""

import time

import numpy as np

import concourse.bacc as bacc
import concourse.mybir as mybir
import concourse.tile as tile

B, O, I, D, M = 64, 32, 512, 64, 64
CORES = 8
IL = I // CORES          # 64 local i per core
J = IL // 2              # 32 i-pairs per core
OD = O * D               # 2048
HALF = OD // 2           # 1024 columns = d 0:32 (d-major layout)
EPS = 1e-8
QW = 4                   # i-pairs per DVE instruction in routing loops

F16 = mybir.dt.float16
F32 = mybir.dt.float32


def _build(debug=False, repeat=1, stage=7, skip_ar=False, ar_chunks=2):
    nc = bacc.Bacc("TRN2", target_bir_lowering=False, debug=False,
                   num_devices=CORES)
    ALU = mybir.AluOpType
    AX = mybir.AxisListType.X
    AF = mybir.ActivationFunctionType

    xt_d = nc.dram_tensor("xt", [128, J * B], F16, kind="ExternalInput").ap()
    wt_d = nc.dram_tensor("wt", [J // 2, 128, 2 * OD], F16, kind="ExternalInput").ap()
    dl_d = nc.dram_tensor("dl", [128, B], F16, kind="ExternalInput").ap()
    s2_d = nc.dram_tensor("s2out", [B, OD], F32, kind="ExternalOutput").ap()

    with tile.TileContext(nc) as tc:
        with (
            tc.tile_pool(name="big", bufs=1) as big,
            tc.tile_pool(name="wp", bufs=2) as wp,
            tc.tile_pool(name="tmp", bufs=2) as tmpp,
            tc.tile_pool(name="small", bufs=1) as small,
            tc.tile_pool(name="scr", bufs=2) as scrp,
            tc.tile_pool(name="stats", bufs=1) as stats,
            tc.tile_pool(name="ppool", bufs=4, space="PSUM") as ppool,
            tc.tile_pool(name="spool", bufs=1, space="PSUM") as spool,
            tc.tile_pool(name="dram", bufs=1, space="DRAM") as dram,
        ):
            xh = big.tile([128, J * OD], F16)          # resident x_hat, fp16
            xall = small.tile([128, J * B], F16, tag="xall")
            dl = small.tile([128, B], F16, tag="dl")
            nc.sync.dma_start(xall[:], xt_d)
            nc.sync.dma_start(dl[:], dl_d)

            sg = small.tile([128, OD], F16, tag="sg")   # all-reduced s, rows 0:64 == 64:128
            ssb = small.tile([B, OD], F32, tag="ssb")   # final out staging
            ssb16 = small.tile([B, OD], F16, tag="ssb16")  # AR staging
            blog = small.tile([128, J * O], F32, tag="blog")  # accumulated logits
            qh = small.tile([128, J * O], F16, tag="qh")      # chunk-b partial dots

            for rep in range(repeat):
                ar_of = {}
                for h in range(2):
                    for it in range(2):
                        w = OD if (ar_chunks == 1 and h == 0) else HALF
                        ai = dram.tile([B, w], F16, tag=f"ar_in{it}{h}_{rep}",
                                       name=f"ar_in{it}{h}_{rep}")
                        ao = dram.tile([B, w], F16, tag=f"ar_out{it}{h}_{rep}",
                                       name=f"ar_out{it}{h}_{rep}", addr_space="Shared")
                        ar_of[(it, h)] = (ai, ao)

                sp = spool.tile([B, OD], F32, tag="sp")

                # ---- production: x_hat tiles + fp16 eviction + iter0 dsum ----
                # dsum0 matmuls are emitted LAG j's behind production so the
                # in-order PE queue never waits on a fresh eviction.
                LAG = 4

                def dsum0_j(j, gs):
                    for g in gs:
                        nc.tensor.matmul(
                            sp[:, g * 512:(g + 1) * 512],
                            dl[:],
                            xh[:, j * OD + g * 512:j * OD + (g + 1) * 512],
                            start=(j == 0), stop=(j == J - 1),
                        )

                ev = 0
                for jp in range(J // 2):
                    w_j = wp.tile([128, 2 * OD], F16, tag="w")
                    nc.sync.dma_start(w_j[:], wt_d[jp])
                    for jq in range(2):
                        j = 2 * jp + jq
                        for half in range(2):
                            for g in range(2):
                                pt = ppool.tile([128, 512], F32, tag="pt")
                                od0 = jq * OD + half * 1024 + g * 512
                                for i2 in range(2):
                                    nc.tensor.matmul(
                                        pt[i2 * 64:(i2 + 1) * 64, :],
                                        xall[i2 * 64:(i2 + 1) * 64,
                                             j * B:(j + 1) * B],
                                        w_j[i2 * 64:(i2 + 1) * 64,
                                            od0:od0 + 512],
                                        start=True, stop=True,
                                        tile_position=(i2 * 64, i2 * 64),
                                    )
                                dst = xh[:, j * OD + half * 1024 + g * 512:
                                         j * OD + half * 1024 + g * 512 + 512]
                                if ev % 2 == 0:
                                    nc.vector.tensor_copy(dst, pt[:])
                                else:
                                    nc.scalar.copy(dst, pt[:])
                                ev += 1
                        if j >= LAG:
                            dsum0_j(j - LAG, (0, 1, 2, 3))
                for j in range(J - LAG, J):
                    dsum0_j(j, (0, 1, 2, 3))

                if stage == 1:
                    nc.vector.tensor_copy(ssb16[:], sp[:])
                    nc.vector.tensor_copy(ssb[:], sp[:])
                    nc.sync.dma_start(s2_d, ssb[:])
                    continue

                # ---- AllReduce d-chunk(s): psum -> sbuf(fp16) -> dram -> CC -> sg
                def allreduce_chunk(it, h, scale, width):
                    c0 = h * HALF
                    if scale == 1.0:
                        nc.scalar.copy(ssb16[:, c0:c0 + width], sp[:, c0:c0 + width])
                    else:
                        nc.scalar.mul(ssb16[:, c0:c0 + width], sp[:, c0:c0 + width],
                                      scale)
                    ai, ao = ar_of[(it, h)]
                    if skip_ar:
                        nc.sync.dma_start(sg[0:64, c0:c0 + width],
                                          ssb16[:, c0:c0 + width])
                        nc.sync.dma_start(sg[64:128, c0:c0 + width],
                                          ssb16[:, c0:c0 + width])
                        return
                    nc.sync.dma_start(ai[:], ssb16[:, c0:c0 + width])
                    nc.gpsimd.collective_compute(
                        "AllReduce", ALU.add,
                        replica_groups=[list(range(CORES))],
                        ins=[ai.opt()], outs=[ao.opt()],
                    )
                    nc.sync.dma_start(sg[0:64, c0:c0 + width], ao[:])
                    nc.sync.dma_start(sg[64:128, c0:c0 + width], ao[:])

                def allreduce_s(it, scale):
                    if ar_chunks == 2:
                        allreduce_chunk(it, 0, scale, HALF)
                        allreduce_chunk(it, 1, scale, HALF)
                    else:
                        allreduce_chunk(it, 0, scale, OD)

                # ---- n2 partial for one d-half: n2 += sum_d sg^2 ------------
                def n2_half(h, n2):
                    c0 = h * HALF
                    sq = scrp.tile([128, HALF], F16, tag="sq")
                    nc.vector.tensor_mul(sq[:], sg[:, c0:c0 + HALF],
                                         sg[:, c0:c0 + HALF])
                    nt = stats.tile([128, O], F32, tag=f"n2h{h}")
                    nc.vector.reduce_sum(nt[:],
                                         sq.rearrange("p (d o) -> p o d", o=O),
                                         axis=AX)
                    if h == 0:
                        nc.vector.tensor_copy(n2[:], nt[:])
                    else:
                        nc.vector.tensor_add(n2[:], n2[:], nt[:])

                # ---- squash factor f = n2 / ((1+n2)(n+eps)), n = sqrt(n2) ---
                def squash_factor(n2, ff):
                    n1 = stats.tile([128, O], F32, tag="n1")
                    # sqrt via exp(0.5*ln(.)): keeps ACT on one table set
                    nc.scalar.activation(n1[:], n2[:], AF.Ln)
                    nc.scalar.activation(n1[:], n1[:], AF.Exp, scale=0.5)
                    t1 = stats.tile([128, O], F32, tag="t1")
                    nc.vector.tensor_scalar_add(t1[:], n2[:], 1.0)
                    nc.vector.reciprocal(t1[:], t1[:])
                    t2 = stats.tile([128, O], F32, tag="t2")
                    nc.vector.tensor_scalar_add(t2[:], n1[:], EPS)
                    nc.vector.reciprocal(t2[:], t2[:])
                    nc.vector.tensor_mul(ff[:], n2[:], t1[:])
                    nc.vector.tensor_mul(ff[:], ff[:], t2[:])

                # ---- dot products q_h[b,(j,o)] = sum_{d in half h} xh*sg ----
                # chunk a (h=0) lands in qa (fp32); chunk b (h=1) folds into
                # qh and is immediately combined into the logits per-jp so the
                # small tail ops pipeline under the next jp's folds:
                #   it==1: blog  = f*(qa+qb);  it==2: blog += f*(qa+qb)
                def dots_half(h, it, ff):
                    c0 = h * HALF
                    for jp in range(J // QW):
                        j0 = QW * jp
                        t = tmpp.tile([128, QW * HALF], F16, tag="tmp4", bufs=2)
                        t3 = t.rearrange("p (j q) -> p j q", q=HALF)
                        nc.vector.tensor_tensor(
                            t3,
                            xh.rearrange("p (j q) -> p j q", q=OD)
                            [:, j0:j0 + QW, c0:c0 + HALF],
                            sg[:, c0:c0 + HALF].unsqueeze(1)
                            .broadcast_to([128, QW, HALF]),
                            ALU.mult,
                        )
                        # fold d 32 -> 1 (d-major layout: halves are contiguous)
                        nc.vector.tensor_add(t3[:, :, 0:512], t3[:, :, 0:512],
                                             t3[:, :, 512:1024])
                        nc.vector.tensor_add(t3[:, :, 0:256], t3[:, :, 0:256],
                                             t3[:, :, 256:512])
                        nc.vector.tensor_add(t3[:, :, 0:128], t3[:, :, 0:128],
                                             t3[:, :, 128:256])
                        nc.vector.tensor_add(t3[:, :, 0:64], t3[:, :, 0:64],
                                             t3[:, :, 64:128])
                        cols = slice(j0 * O, (j0 + QW) * O)
                        dst = qa if h == 0 else qh
                        nc.vector.tensor_add(
                            dst[:, cols].rearrange("p (j o) -> p j o", o=O),
                            t3[:, :, 0:32], t3[:, :, 32:64])
                    if h == 1:
                        # combine once at full width: fewer DVE ops beats
                        # per-block tails (all of this is serial on the DVE
                        # queue either way)
                        nc.vector.tensor_add(qa[:], qa[:], qh[:])
                        q3 = qa.rearrange("p (j o) -> p j o", o=O)
                        fb = ff.unsqueeze(1).broadcast_to([128, J, O])
                        if it == 1:
                            nc.vector.tensor_tensor(
                                blog.rearrange("p (j o) -> p j o", o=O),
                                q3, fb, ALU.mult)
                        else:
                            nc.vector.tensor_tensor(q3, q3, fb, ALU.mult)
                            nc.vector.tensor_add(blog[:], blog[:], qa[:])

                # ---- softmax over o (innermost of (j,o)) -> cc fp16 ----------
                def softmax(cc):
                    # no max-subtraction: logits are bounded (|b| < ~20); exp
                    # is exact enough in fp32
                    ee = scrp.tile([128, J * O], F32, tag="scr")
                    ee3 = ee.rearrange("p (j o) -> p j o", o=O)
                    nc.scalar.activation(ee[:], blog[:], AF.Exp)
                    den = stats.tile([128, J], F32, tag="den")
                    nc.vector.reduce_sum(den[:], ee3, axis=AX)
                    nc.vector.reciprocal(den[:], den[:])
                    nc.vector.tensor_tensor(
                        cc.rearrange("p (j o) -> p j o", o=O), ee3,
                        den.unsqueeze(2).broadcast_to([128, J, O]), ALU.mult)

                # ---- weighted sum for d-half h: DVE mult + PE dsum ----------
                def weighted_half(cc, h):
                    c0 = h * HALF
                    for jp in range(J // QW):
                        j0 = QW * jp
                        xc = tmpp.tile([128, QW * HALF], F16, tag="tmp4", bufs=2)
                        nc.vector.tensor_tensor(
                            xc.rearrange("p (j d o) -> p j d o", d=D // 2, o=O),
                            xh.rearrange("p (j q) -> p j q", q=OD)
                            [:, j0:j0 + QW, c0:c0 + HALF]
                            .rearrange("p j (d o) -> p j d o", o=O),
                            cc[:, j0 * O:(j0 + QW) * O]
                            .rearrange("p (j o) -> p j o", o=O)
                            .unsqueeze(2).broadcast_to([128, QW, D // 2, O]),
                            ALU.mult,
                        )
                        for q in range(QW):
                            for g in range(2):
                                nc.tensor.matmul(
                                    sp[:, c0 + g * 512:c0 + (g + 1) * 512],
                                    dl[:],
                                    xc[:, q * HALF + g * 512:
                                       q * HALF + (g + 1) * 512],
                                    start=(jp == 0 and q == 0),
                                    stop=(jp == J // QW - 1 and q == QW - 1),
                                )

                # ================= routing =================
                # iteration 0: c uniform -> s0 = (1/32) sum_i x_hat (in sp)
                n2 = stats.tile([128, O], F32, tag="n2")
                ff = stats.tile([128, O], F32, tag="ff")
                qa = scrp.tile([128, J * O], F32, tag="scr")

                for it in (1, 2):
                    sc = (1.0 / O) if it == 1 else 1.0
                    allreduce_s(it - 1, sc)
                    n2_half(0, n2)
                    dots_half(0, it, ff)
                    n2_half(1, n2)
                    squash_factor(n2, ff)
                    dots_half(1, it, ff)
                    if stage == 2 * it:   # logits after iteration `it`
                        nc.vector.tensor_copy(ssb16[:, 0:J * O], blog[0:64, :])
                        nc.vector.tensor_copy(ssb[:, 0:J * O], blog[0:64, :])
                        nc.sync.dma_start(s2_d, ssb[:])
                        break
                    cc = scrp.tile([128, J * O], F16, tag="cc")
                    softmax(cc)
                    weighted_half(cc, 0)
                    if it == 2:
                        # stream the output out per d-half as its dsums stop
                        nc.scalar.copy(ssb[:, 0:HALF], sp[:, 0:HALF])
                        nc.sync.dma_start(s2_d[:, 0:HALF], ssb[:, 0:HALF])
                    weighted_half(cc, 1)
                    if stage == 2 * it + 1:
                        nc.vector.tensor_copy(ssb16[:], sp[:])
                        nc.vector.tensor_copy(ssb[:], sp[:])
                        nc.sync.dma_start(s2_d, ssb[:])
                        break
                    if it == 2:
                        nc.scalar.copy(ssb[:, HALF:OD], sp[:, HALF:OD])
                        nc.sync.dma_start(s2_d[:, HALF:OD], ssb[:, HALF:OD])

    nc.compile()
    return nc


def _prep(x, weight):
    """Host-side shard + relayout + fp16 cast."""
    x16 = x.astype(np.float16)
    w16 = weight.astype(np.float16)
    xs, ws = [], []
    for c in range(CORES):
        xc = x16[:, c * IL:(c + 1) * IL, :]                 # [B, IL, M]
        xc = xc.reshape(B, J, 2, M).transpose(2, 3, 1, 0)   # [i2, m, j, b]
        xs.append(np.ascontiguousarray(xc.reshape(128, J * B)))
        wc = w16[:, c * IL:(c + 1) * IL, :, :]              # [O, IL, D, M]
        # [o, jp, jq, i2, d, m] -> [jp, i2, m, jq, d, o]
        wc = wc.reshape(O, J // 2, 2, 2, D, M).transpose(1, 3, 5, 2, 4, 0)
        ws.append(np.ascontiguousarray(wc.reshape(J // 2, 128, 2 * OD)))
    dl = np.concatenate([np.eye(B, dtype=np.float16)] * 2, axis=0)  # [128, B]
    return xs, ws, dl


def _squash_np(v):
    n = np.linalg.norm(v, axis=-1, keepdims=True)
    n2 = n * n
    return (n2 / (1.0 + n2)) * v / (n + EPS)


class _Runner:
    """Compile once, execute many times.

    Mirrors the multi-core axon branch of
    concourse.bass_utils.run_bass_kernel_spmd (which lowers through
    bass2jax.run_bass_via_pjrt), but keeps the jitted executable alive so
    repeated calls don't retrace/recompile.
    """

    def __init__(self, nc):
        import jax
        import jax.numpy as jnp  # noqa: F401
        from jax.sharding import Mesh, PartitionSpec
        from jax.experimental.shard_map import shard_map
        from concourse import bass2jax
        from concourse.bass2jax import install_neuronx_cc_hook

        install_neuronx_cc_hook()
        self.nc = nc
        partition_name = (nc.partition_id_tensor.name
                          if nc.partition_id_tensor else None)
        in_names, out_names, out_avals, zero_outs = [], [], [], []
        for alloc in nc.m.functions[0].allocations:
            if not isinstance(alloc, mybir.MemoryLocationSet):
                continue
            name = alloc.memorylocations[0].name
            if alloc.kind == "ExternalInput":
                if name != partition_name:
                    in_names.append(name)
            elif alloc.kind == "ExternalOutput":
                out_names.append(name)
                shape = tuple(alloc.tensor_shape)
                dtype = mybir.dt.np(alloc.dtype)
                out_avals.append(jax.core.ShapedArray(shape, dtype))
                zero_outs.append(np.zeros(shape, dtype))
        n_params = len(in_names)
        n_outs = len(out_avals)
        all_in_names = list(in_names) + list(out_names)
        if partition_name is not None:
            all_in_names.append(partition_name)
        self.in_names = in_names
        self.out_names = out_names
        self.zero_outs = zero_outs
        self.out_avals = out_avals

        def _body(*args):
            operands = list(args)
            if partition_name is not None:
                operands.append(bass2jax.partition_id_tensor())
            outs = bass2jax._bass_exec_p.bind(
                *operands,
                out_avals=tuple(out_avals),
                in_names=tuple(all_in_names),
                out_names=tuple(out_names),
                lowering_input_output_aliases=(),
                sim_require_finite=True,
                sim_require_nnan=True,
                nc=nc,
            )
            return tuple(outs)

        devices = jax.devices()[:CORES]
        assert len(devices) == CORES
        mesh = Mesh(np.asarray(devices), ("core",))
        in_specs = (PartitionSpec("core"),) * (n_params + n_outs)
        out_specs = (PartitionSpec("core"),) * n_outs
        donate = tuple(range(n_params, n_params + n_outs))
        self.sharded = jax.jit(
            shard_map(_body, mesh=mesh, in_specs=in_specs,
                      out_specs=out_specs, check_rep=False),
            donate_argnums=donate, keep_unused=True,
        )

    def __call__(self, in_maps):
        concat_in = [
            np.concatenate([np.asarray(m[name]) for m in in_maps], axis=0)
            for name in self.in_names
        ]
        concat_zeros = [
            np.zeros((CORES * z.shape[0], *z.shape[1:]), z.dtype)
            for z in self.zero_outs
        ]
        out_arrs = self.sharded(*concat_in, *concat_zeros)
        return [
            {
                name: np.asarray(out_arrs[i]).reshape(
                    CORES, *self.out_avals[i].shape)[c]
                for i, name in enumerate(self.out_names)
            }
            for c in range(CORES)
        ]


_RUNNERS = {}


def _get_runner(debug=False, repeat=1, stage=7, skip_ar=False, ar_chunks=2):
    key = (debug, repeat, stage, skip_ar, ar_chunks)
    if key not in _RUNNERS:
        _RUNNERS[key] = _Runner(_build(debug, repeat, stage, skip_ar, ar_chunks))
    return _RUNNERS[key]


def make_in_maps(x, weight):
    xs, ws, dl = _prep(np.asarray(x, np.float32), np.asarray(weight, np.float32))
    return [{"xt": xs[c], "wt": ws[c], "dl": dl} for c in range(CORES)]


def finish(results):
    s2 = np.zeros((B, OD), np.float32)
    for c in range(CORES):
        s2 += results[c]["s2out"]
    s2 = s2.reshape(B, D, O).transpose(0, 2, 1)
    return _squash_np(s2).astype(np.float32)


def kernel(x, weight):
    runner = _get_runner(debug=False)
    results = runner(make_in_maps(x, weight))
    return finish(results)


if __name__ == "__main__":
    rng = np.random.default_rng(0)
    x = rng.standard_normal((B, I, M)).astype(np.float32)
    w = rng.standard_normal((O, I, D, M)).astype(np.float32) * 0.1
    t0 = time.time()
    out = kernel(x, w)
    print("first call (incl compile):", time.time() - t0, "s; out", out.shape)


# revision 30
# speedup vs baseline: 1.3678x; 1.3678x over previous
"""CapsuleLayer (dynamic routing, 3 iterations) on 8 Trainium2 NeuronCores.

Math (see reference):
    x_hat[b,o,i,d] = sum_m W[o,i,d,m] * x[b,i,m]
    b_log = 0; for it in 0..2:
        c = softmax(b_log, axis=o)
        s = sum_i c[b,o,i] * x_hat[b,o,i,d]; out = squash(s)
        if it < 2: b_log += x_hat . out

Sharding: split I=512 over 8 cores (64 i's each).  The weight (the only
large operand, 134MB fp16) is then read exactly once from HBM.  The softmax
over o is local; only s[b,o,d] needs a cross-core AllReduce (3 sums total:
the first two on-device, the last one on the host as part of unsharding).

Per-core layout: x_hat resident in SBUF as fp16, 32 "i-pair" tiles
[128=(i2,b), (d,o)=2048] (d-major, o-minor: the big tensor_tensor
multiplies broadcast per-(o) vectors over d with a packed unit-stride
innermost o, which qualifies them for the DVE 2x 16-bit perf mode).

Key structure vs a naive mapping:
  - production matmuls use two concurrent 64x64 PE quadrants
    (tile_position) per i-pair; PSUM eviction tiles are [128,512] with 4
    buffers so the fp32->fp16 eviction copies run on DVE and ACT in
    parallel without pacing the PE.
  - the agreement logits use the identity  x_hat.out = f * (x_hat.s)
    where out = squash(s) = f(|s|)*s, so the dot products run against the
    raw AllReduced s (sg) and the scalar f is applied after the
    d-reduction on the small [128, j*o] tile.  This removes the full-size
    squash multiply and lets each d-half of the dot products start as
    soon as its AllReduce chunk lands.
  - s sums over i run on the PE with a block-diagonal [eye;eye]
    stationary; weighted sums (iters 1/2) are DVE multiplies (2x mode)
    chased by the PE, d-half by d-half, with the AllReduce of half 0
    overlapped against the compute of half 1.
  - all activation functions are from one ACT table set (exp/ln);
    sqrt(n2) is computed as exp(0.5*ln(n2)) to avoid ~2.7us table
    reloads between squash and softmax.
"""

import time

import numpy as np

import concourse.bacc as bacc
import concourse.mybir as mybir
import concourse.tile as tile

B, O, I, D, M = 64, 32, 512, 64, 64
CORES = 8
IL = I // CORES          # 64 local i per core
J = IL // 2              # 32 i-pairs per core
OD = O * D               # 2048
HALF = OD // 2           # 1024 columns = d 0:32 (d-major layout)
EPS = 1e-8
QW = 4                   # i-pairs per DVE instruction in routing loops

F16 = mybir.dt.float16
F32 = mybir.dt.float32


def _build(debug=False, repeat=1, stage=7, skip_ar=False, ar_chunks=2):
    nc = bacc.Bacc("TRN2", target_bir_lowering=False, debug=False,
                   num_devices=CORES)
    ALU = mybir.AluOpType
    AX = mybir.AxisListType.X
    AF = mybir.ActivationFunctionType

    xt_d = nc.dram_tensor("xt", [128, J * B], F16, kind="ExternalInput").ap()
    wt_d = nc.dram_tensor("wt", [J // 2, 128, 2 * OD], F16, kind="ExternalInput").ap()
    dl_d = nc.dram_tensor("dl", [128, B], F16, kind="ExternalInput").ap()
    s2_d = nc.dram_tensor("s2out", [B, OD], F32, kind="ExternalOutput").ap()

    with tile.TileContext(nc) as tc:
        with (
            tc.tile_pool(name="big", bufs=1) as big,
            tc.tile_pool(name="wp", bufs=2) as wp,
            tc.tile_pool(name="tmp", bufs=2) as tmpp,
            tc.tile_pool(name="small", bufs=1) as small,
            tc.tile_pool(name="scr", bufs=2) as scrp,
            tc.tile_pool(name="stats", bufs=1) as stats,
            tc.tile_pool(name="ppool", bufs=4, space="PSUM") as ppool,
            tc.tile_pool(name="spool", bufs=1, space="PSUM") as spool,
            tc.tile_pool(name="dram", bufs=1, space="DRAM") as dram,
        ):
            xh = big.tile([128, J * OD], F16)          # resident x_hat, fp16
            xall = small.tile([128, J * B], F16, tag="xall")
            dl = small.tile([128, B], F16, tag="dl")
            nc.sync.dma_start(xall[:], xt_d)
            nc.sync.dma_start(dl[:], dl_d)

            sg = small.tile([128, OD], F16, tag="sg")   # all-reduced s, rows 0:64 == 64:128
            ssb = small.tile([B, OD], F32, tag="ssb")   # final out staging
            ssb16 = small.tile([B, OD], F16, tag="ssb16")  # AR staging
            blog = small.tile([128, J * O], F32, tag="blog")  # accumulated logits
            qh = small.tile([128, J * O], F16, tag="qh")      # chunk-b partial dots

            for rep in range(repeat):
                ar_of = {}
                for h in range(2):
                    for it in range(2):
                        w = OD if (ar_chunks == 1 and h == 0) else HALF
                        ai = dram.tile([B, w], F16, tag=f"ar_in{it}{h}_{rep}",
                                       name=f"ar_in{it}{h}_{rep}")
                        ao = dram.tile([B, w], F16, tag=f"ar_out{it}{h}_{rep}",
                                       name=f"ar_out{it}{h}_{rep}", addr_space="Shared")
                        ar_of[(it, h)] = (ai, ao)

                sp = spool.tile([B, OD], F32, tag="sp")

                # ---- production: x_hat tiles + fp16 eviction + iter0 dsum ----
                # dsum0 matmuls are emitted LAG j's behind production so the
                # in-order PE queue never waits on a fresh eviction.
                LAG = 4

                def dsum0_j(j, gs):
                    for g in gs:
                        nc.tensor.matmul(
                            sp[:, g * 512:(g + 1) * 512],
                            dl[:],
                            xh[:, j * OD + g * 512:j * OD + (g + 1) * 512],
                            start=(j == 0), stop=(j == J - 1),
                        )

                ev = 0
                for jp in range(J // 2):
                    w_j = wp.tile([128, 2 * OD], F16, tag="w")
                    nc.sync.dma_start(w_j[:], wt_d[jp])
                    for jq in range(2):
                        j = 2 * jp + jq
                        for half in range(2):
                            for g in range(2):
                                pt = ppool.tile([128, 512], F32, tag="pt")
                                od0 = jq * OD + half * 1024 + g * 512
                                for i2 in range(2):
                                    nc.tensor.matmul(
                                        pt[i2 * 64:(i2 + 1) * 64, :],
                                        xall[i2 * 64:(i2 + 1) * 64,
                                             j * B:(j + 1) * B],
                                        w_j[i2 * 64:(i2 + 1) * 64,
                                            od0:od0 + 512],
                                        start=True, stop=True,
                                        tile_position=(i2 * 64, i2 * 64),
                                    )
                                dst = xh[:, j * OD + half * 1024 + g * 512:
                                         j * OD + half * 1024 + g * 512 + 512]
                                if ev % 2 == 0:
                                    nc.vector.tensor_copy(dst, pt[:])
                                else:
                                    nc.scalar.copy(dst, pt[:])
                                ev += 1
                        if j >= LAG:
                            dsum0_j(j - LAG, (0, 1, 2, 3))
                for j in range(J - LAG, J):
                    dsum0_j(j, (0, 1, 2, 3))

                if stage == 1:
                    nc.vector.tensor_copy(ssb16[:], sp[:])
                    nc.vector.tensor_copy(ssb[:], sp[:])
                    nc.sync.dma_start(s2_d, ssb[:])
                    continue

                # ---- AllReduce d-chunk(s): psum -> sbuf(fp16) -> dram -> CC -> sg
                def allreduce_chunk(it, h, scale, width):
                    c0 = h * HALF
                    if scale == 1.0:
                        nc.scalar.copy(ssb16[:, c0:c0 + width], sp[:, c0:c0 + width])
                    else:
                        nc.scalar.mul(ssb16[:, c0:c0 + width], sp[:, c0:c0 + width],
                                      scale)
                    ai, ao = ar_of[(it, h)]
                    if skip_ar:
                        nc.sync.dma_start(sg[0:64, c0:c0 + width],
                                          ssb16[:, c0:c0 + width])
                        nc.sync.dma_start(sg[64:128, c0:c0 + width],
                                          ssb16[:, c0:c0 + width])
                        return
                    nc.sync.dma_start(ai[:], ssb16[:, c0:c0 + width])
                    nc.gpsimd.collective_compute(
                        "AllReduce", ALU.add,
                        replica_groups=[list(range(CORES))],
                        ins=[ai.opt()], outs=[ao.opt()],
                    )
                    nc.sync.dma_start(sg[0:64, c0:c0 + width], ao[:])
                    nc.sync.dma_start(sg[64:128, c0:c0 + width], ao[:])

                def allreduce_s(it, scale):
                    if ar_chunks == 2:
                        allreduce_chunk(it, 0, scale, HALF)
                        allreduce_chunk(it, 1, scale, HALF)
                    else:
                        allreduce_chunk(it, 0, scale, OD)

                # ---- n2 partial for one d-half: n2 += sum_d sg^2 ------------
                def n2_half(h, n2):
                    c0 = h * HALF
                    sq = scrp.tile([128, HALF], F16, tag="sq")
                    nc.vector.tensor_mul(sq[:], sg[:, c0:c0 + HALF],
                                         sg[:, c0:c0 + HALF])
                    nt = stats.tile([128, O], F32, tag=f"n2h{h}")
                    nc.vector.reduce_sum(nt[:],
                                         sq.rearrange("p (d o) -> p o d", o=O),
                                         axis=AX)
                    if h == 0:
                        nc.vector.tensor_copy(n2[:], nt[:])
                    else:
                        nc.vector.tensor_add(n2[:], n2[:], nt[:])

                # ---- squash factor f = n2 / ((1+n2)(n+eps)), n = sqrt(n2) ---
                def squash_factor(n2, ff):
                    n1 = stats.tile([128, O], F32, tag="n1")
                    # sqrt via exp(0.5*ln(.)): keeps ACT on one table set
                    nc.scalar.activation(n1[:], n2[:], AF.Ln)
                    nc.scalar.activation(n1[:], n1[:], AF.Exp, scale=0.5)
                    t1 = stats.tile([128, O], F32, tag="t1")
                    nc.vector.tensor_scalar_add(t1[:], n2[:], 1.0)
                    nc.vector.reciprocal(t1[:], t1[:])
                    t2 = stats.tile([128, O], F32, tag="t2")
                    nc.vector.tensor_scalar_add(t2[:], n1[:], EPS)
                    nc.vector.reciprocal(t2[:], t2[:])
                    nc.vector.tensor_mul(ff[:], n2[:], t1[:])
                    nc.vector.tensor_mul(ff[:], ff[:], t2[:])

                # ---- dot products q_h[b,(j,o)] = sum_{d in half h} xh*sg ----
                # chunk a (h=0) lands in qa (fp32); chunk b (h=1) folds into
                # qh and is immediately combined into the logits per-jp so the
                # small tail ops pipeline under the next jp's folds:
                #   it==1: blog  = f*(qa+qb);  it==2: blog += f*(qa+qb)
                def dots_half(h, it, ff):
                    c0 = h * HALF
                    for jp in range(J // QW):
                        j0 = QW * jp
                        t = tmpp.tile([128, QW * HALF], F16, tag="tmp4", bufs=2)
                        t3 = t.rearrange("p (j q) -> p j q", q=HALF)
                        nc.vector.tensor_tensor(
                            t3,
                            xh.rearrange("p (j q) -> p j q", q=OD)
                            [:, j0:j0 + QW, c0:c0 + HALF],
                            sg[:, c0:c0 + HALF].unsqueeze(1)
                            .broadcast_to([128, QW, HALF]),
                            ALU.mult,
                        )
                        # fold d 32 -> 1 (d-major layout: halves are contiguous)
                        nc.vector.tensor_add(t3[:, :, 0:512], t3[:, :, 0:512],
                                             t3[:, :, 512:1024])
                        nc.vector.tensor_add(t3[:, :, 0:256], t3[:, :, 0:256],
                                             t3[:, :, 256:512])
                        nc.vector.tensor_add(t3[:, :, 0:128], t3[:, :, 0:128],
                                             t3[:, :, 128:256])
                        nc.vector.tensor_add(t3[:, :, 0:64], t3[:, :, 0:64],
                                             t3[:, :, 64:128])
                        cols = slice(j0 * O, (j0 + QW) * O)
                        dst = qa if h == 0 else qh
                        nc.vector.tensor_add(
                            dst[:, cols].rearrange("p (j o) -> p j o", o=O),
                            t3[:, :, 0:32], t3[:, :, 32:64])
                    if h == 1:
                        # combine once at full width: fewer DVE ops beats
                        # per-block tails (all of this is serial on the DVE
                        # queue either way)
                        nc.vector.tensor_add(qa[:], qa[:], qh[:])
                        q3 = qa.rearrange("p (j o) -> p j o", o=O)
                        fb = ff.unsqueeze(1).broadcast_to([128, J, O])
                        if it == 1:
                            nc.vector.tensor_tensor(
                                blog.rearrange("p (j o) -> p j o", o=O),
                                q3, fb, ALU.mult)
                        else:
                            nc.vector.tensor_tensor(q3, q3, fb, ALU.mult)
                            nc.vector.tensor_add(blog[:], blog[:], qa[:])

                # ---- softmax over o (innermost of (j,o)) -> cc fp16 ----------
                def softmax(cc):
                    # no max-subtraction: logits are bounded (|b| < ~20); exp
                    # is exact enough in fp32
                    ee = scrp.tile([128, J * O], F32, tag="scr")
                    ee3 = ee.rearrange("p (j o) -> p j o", o=O)
                    nc.scalar.activation(ee[:], blog[:], AF.Exp)
                    den = stats.tile([128, J], F32, tag="den")
                    nc.vector.reduce_sum(den[:], ee3, axis=AX)
                    nc.vector.reciprocal(den[:], den[:])
                    nc.vector.tensor_tensor(
                        cc.rearrange("p (j o) -> p j o", o=O), ee3,
                        den.unsqueeze(2).broadcast_to([128, J, O]), ALU.mult)

                # ---- weighted sum for d-half h: DVE mult + PE dsum ----------
                def weighted_half(cc, h):
                    c0 = h * HALF
                    for jp in range(J // QW):
                        j0 = QW * jp
                        xc = tmpp.tile([128, QW * HALF], F16, tag="tmp4", bufs=2)
                        nc.vector.tensor_tensor(
                            xc.rearrange("p (j d o) -> p j d o", d=D // 2, o=O),
                            xh.rearrange("p (j q) -> p j q", q=OD)
                            [:, j0:j0 + QW, c0:c0 + HALF]
                            .rearrange("p j (d o) -> p j d o", o=O),
                            cc[:, j0 * O:(j0 + QW) * O]
                            .rearrange("p (j o) -> p j o", o=O)
                            .unsqueeze(2).broadcast_to([128, QW, D // 2, O]),
                            ALU.mult,
                        )
                        for q in range(QW):
                            for g in range(2):
                                nc.tensor.matmul(
                                    sp[:, c0 + g * 512:c0 + (g + 1) * 512],
                                    dl[:],
                                    xc[:, q * HALF + g * 512:
                                       q * HALF + (g + 1) * 512],
                                    start=(jp == 0 and q == 0),
                                    stop=(jp == J // QW - 1 and q == QW - 1),
                                )

                # ================= routing =================
                # iteration 0: c uniform -> s0 = (1/32) sum_i x_hat (in sp)
                n2 = stats.tile([128, O], F32, tag="n2")
                ff = stats.tile([128, O], F32, tag="ff")
                qa = scrp.tile([128, J * O], F32, tag="scr")

                for it in (1, 2):
                    sc = (1.0 / O) if it == 1 else 1.0
                    allreduce_s(it - 1, sc)
                    n2_half(0, n2)
                    dots_half(0, it, ff)
                    n2_half(1, n2)
                    squash_factor(n2, ff)
                    dots_half(1, it, ff)
                    if stage == 2 * it:   # logits after iteration `it`
                        nc.vector.tensor_copy(ssb16[:, 0:J * O], blog[0:64, :])
                        nc.vector.tensor_copy(ssb[:, 0:J * O], blog[0:64, :])
                        nc.sync.dma_start(s2_d, ssb[:])
                        break
                    cc = scrp.tile([128, J * O], F16, tag="cc")
                    softmax(cc)
                    weighted_half(cc, 0)
                    if it == 2:
                        # stream the output out per d-half as its dsums stop
                        nc.scalar.copy(ssb[:, 0:HALF], sp[:, 0:HALF])
                        nc.sync.dma_start(s2_d[:, 0:HALF], ssb[:, 0:HALF])
                    weighted_half(cc, 1)
                    if stage == 2 * it + 1:
                        nc.vector.tensor_copy(ssb16[:], sp[:])
                        nc.vector.tensor_copy(ssb[:], sp[:])
                        nc.sync.dma_start(s2_d, ssb[:])
                        break
                    if it == 2:
                        nc.scalar.copy(ssb[:, HALF:OD], sp[:, HALF:OD])
                        nc.sync.dma_start(s2_d[:, HALF:OD], ssb[:, HALF:OD])

    nc.compile()
    return nc


def _prep(x, weight):
    """Host-side shard + relayout + fp16 cast."""
    x16 = x.astype(np.float16)
    w16 = weight.astype(np.float16)
    xs, ws = [], []
    for c in range(CORES):
        xc = x16[:, c * IL:(c + 1) * IL, :]                 # [B, IL, M]
        xc = xc.reshape(B, J, 2, M).transpose(2, 3, 1, 0)   # [i2, m, j, b]
        xs.append(np.ascontiguousarray(xc.reshape(128, J * B)))
        wc = w16[:, c * IL:(c + 1) * IL, :, :]              # [O, IL, D, M]
        # [o, jp, jq, i2, d, m] -> [jp, i2, m, jq, d, o]
        wc = wc.reshape(O, J // 2, 2, 2, D, M).transpose(1, 3, 5, 2, 4, 0)
        ws.append(np.ascontiguousarray(wc.reshape(J // 2, 128, 2 * OD)))
    dl = np.concatenate([np.eye(B, dtype=np.float16)] * 2, axis=0)  # [128, B]
    return xs, ws, dl


def _squash_np(v):
    n = np.linalg.norm(v, axis=-1, keepdims=True)
    n2 = n * n
    return (n2 / (1.0 + n2)) * v / (n + EPS)


class _Runner:
    """Compile once, execute many times.

    Mirrors the multi-core axon branch of
    concourse.bass_utils.run_bass_kernel_spmd (which lowers through
    bass2jax.run_bass_via_pjrt), but keeps the jitted executable alive so
    repeated calls don't retrace/recompile.
    """

    def __init__(self, nc):
        import jax
        import jax.numpy as jnp  # noqa: F401
        from jax.sharding import Mesh, PartitionSpec
        from jax.experimental.shard_map import shard_map
        from concourse import bass2jax
        from concourse.bass2jax import install_neuronx_cc_hook

        install_neuronx_cc_hook()
        self.nc = nc
        partition_name = (nc.partition_id_tensor.name
                          if nc.partition_id_tensor else None)
        in_names, out_names, out_avals, zero_outs = [], [], [], []
        for alloc in nc.m.functions[0].allocations:
            if not isinstance(alloc, mybir.MemoryLocationSet):
                continue
            name = alloc.memorylocations[0].name
            if alloc.kind == "ExternalInput":
                if name != partition_name:
                    in_names.append(name)
            elif alloc.kind == "ExternalOutput":
                out_names.append(name)
                shape = tuple(alloc.tensor_shape)
                dtype = mybir.dt.np(alloc.dtype)
                out_avals.append(jax.core.ShapedArray(shape, dtype))
                zero_outs.append(np.zeros(shape, dtype))
        n_params = len(in_names)
        n_outs = len(out_avals)
        all_in_names = list(in_names) + list(out_names)
        if partition_name is not None:
            all_in_names.append(partition_name)
        self.in_names = in_names
        self.out_names = out_names
        self.zero_outs = zero_outs
        self.out_avals = out_avals

        def _body(*args):
            operands = list(args)
            if partition_name is not None:
                operands.append(bass2jax.partition_id_tensor())
            outs = bass2jax._bass_exec_p.bind(
                *operands,
                out_avals=tuple(out_avals),
                in_names=tuple(all_in_names),
                out_names=tuple(out_names),
                lowering_input_output_aliases=(),
                sim_require_finite=True,
                sim_require_nnan=True,
                nc=nc,
            )
            return tuple(outs)

        devices = jax.devices()[:CORES]
        assert len(devices) == CORES
        mesh = Mesh(np.asarray(devices), ("core",))
        in_specs = (PartitionSpec("core"),) * (n_params + n_outs)
        out_specs = (PartitionSpec("core"),) * n_outs
        donate = tuple(range(n_params, n_params + n_outs))
        self.sharded = jax.jit(
            shard_map(_body, mesh=mesh, in_specs=in_specs,
                      out_specs=out_specs, check_rep=False),
            donate_argnums=donate, keep_unused=True,
        )

    def __call__(self, in_maps):
        concat_in = [
            np.concatenate([np.asarray(m[name]) for m in in_maps], axis=0)
            for name in self.in_names
        ]
        concat_zeros = [
            np.zeros((CORES * z.shape[0], *z.shape[1:]), z.dtype)
            for z in self.zero_outs
        ]
        out_arrs = self.sharded(*concat_in, *concat_zeros)
        return [
            {
                name: np.asarray(out_arrs[i]).reshape(
                    CORES, *self.out_avals[i].shape)[c]
                for i, name in enumerate(self.out_names)
            }
            for c in range(CORES)
        ]


_RUNNERS = {}


def _get_runner(debug=False, repeat=1, stage=7, skip_ar=False, ar_chunks=2):
    key = (debug, repeat, stage, skip_ar, ar_chunks)
    if key not in _RUNNERS:
        _RUNNERS[key] = _Runner(_build(debug, repeat, stage, skip_ar, ar_chunks))
    return _RUNNERS[key]


def make_in_maps(x, weight):
    xs, ws, dl = _prep(np.asarray(x, np.float32), np.asarray(weight, np.float32))
    return [{"xt": xs[c], "wt": ws[c], "dl": dl} for c in range(CORES)]


def finish(results):
    s2 = np.zeros((B, OD), np.float32)
    for c in range(CORES):
        s2 += results[c]["s2out"]
    s2 = s2.reshape(B, D, O).transpose(0, 2, 1)
    return _squash_np(s2).astype(np.float32)


def kernel(x, weight):
    runner = _get_runner(debug=False)
    results = runner(make_in_maps(x, weight))
    return finish(results)


if __name__ == "__main__":
    rng = np.random.default_rng(0)
    x = rng.standard_normal((B, I, M)).astype(np.float32)
    w = rng.standard_normal((O, I, D, M)).astype(np.float32) * 0.1
    t0 = time.time()
    out = kernel(x, w)
    print("first call (incl compile):", time.time() - t0, "s; out", out.shape)


# revision 32
# speedup vs baseline: 1.5011x; 1.0975x over previous
"""CapsuleLayer (dynamic routing, 3 iterations) on 8 Trainium2 NeuronCores.

Math (see reference):
    x_hat[b,o,i,d] = sum_m W[o,i,d,m] * x[b,i,m]
    b_log = 0; for it in 0..2:
        c = softmax(b_log, axis=o)
        s = sum_i c[b,o,i] * x_hat[b,o,i,d]; out = squash(s)
        if it < 2: b_log += x_hat . out

Sharding: split I=512 over 8 cores (64 i's each).  The weight (the only
large operand, 134MB fp16) is then read exactly once from HBM.  The softmax
over o is local; only s[b,o,d] needs a cross-core AllReduce (3 sums total:
the first two on-device, the last one on the host as part of unsharding).

Per-core layout: x_hat resident in SBUF as fp16, 32 "i-pair" tiles
[128=(i2,b), (d,o)=2048] (d-major, o-minor: the big tensor_tensor
multiplies broadcast per-(o) vectors over d with a packed unit-stride
innermost o, which qualifies them for the DVE 2x 16-bit perf mode).

Key structure vs a naive mapping:
  - production matmuls use two concurrent 64x64 PE quadrants
    (tile_position) per i-pair; PSUM eviction tiles are [128,512] with 4
    buffers so the fp32->fp16 eviction copies run on DVE and ACT in
    parallel without pacing the PE.
  - the agreement logits use the identity  x_hat.out = f * (x_hat.s)
    where out = squash(s) = f(|s|)*s, so the dot products run against the
    raw AllReduced s (sg) and the scalar f is applied after the
    d-reduction on the small [128, j*o] tile.  This removes the full-size
    squash multiply and lets each d-half of the dot products start as
    soon as its AllReduce chunk lands.
  - s sums over i run on the PE with a block-diagonal [eye;eye]
    stationary; weighted sums (iters 1/2) are DVE multiplies (2x mode)
    chased by the PE, d-half by d-half, with the AllReduce of half 0
    overlapped against the compute of half 1.
  - all activation functions are from one ACT table set (exp/ln);
    sqrt(n2) is computed as exp(0.5*ln(n2)) to avoid ~2.7us table
    reloads between squash and softmax.
"""

import time

import numpy as np

import concourse.bacc as bacc
import concourse.mybir as mybir
import concourse.tile as tile

B, O, I, D, M = 64, 32, 512, 64, 64
CORES = 8
IL = I // CORES          # 64 local i per core
J = IL // 2              # 32 i-pairs per core
OD = O * D               # 2048
HALF = OD // 2           # 1024 columns = d 0:32 (d-major layout)
EPS = 1e-8
QW = 4                   # i-pairs per DVE instruction in routing loops

F16 = mybir.dt.float16
F32 = mybir.dt.float32


def _build(debug=False, repeat=1, stage=7, skip_ar=False, ar_chunks=2):
    nc = bacc.Bacc("TRN2", target_bir_lowering=False, debug=False,
                   num_devices=CORES)
    ALU = mybir.AluOpType
    AX = mybir.AxisListType.X
    AF = mybir.ActivationFunctionType

    xt_d = nc.dram_tensor("xt", [128, J * B], F16, kind="ExternalInput").ap()
    wt_d = nc.dram_tensor("wt", [J, 128, OD], F16, kind="ExternalInput").ap()
    dl_d = nc.dram_tensor("dl", [128, B], F16, kind="ExternalInput").ap()
    s2_d = nc.dram_tensor("s2out", [B, OD], F16, kind="ExternalOutput").ap()

    with tile.TileContext(nc) as tc:
        with (
            tc.tile_pool(name="big", bufs=1) as big,
            tc.tile_pool(name="wp", bufs=2) as wp,
            tc.tile_pool(name="tmp", bufs=2) as tmpp,
            tc.tile_pool(name="small", bufs=1) as small,
            tc.tile_pool(name="scr", bufs=2) as scrp,
            tc.tile_pool(name="stats", bufs=1) as stats,
            tc.tile_pool(name="ppool", bufs=4, space="PSUM") as ppool,
            tc.tile_pool(name="spool", bufs=1, space="PSUM") as spool,
            tc.tile_pool(name="dram", bufs=1, space="DRAM") as dram,
        ):
            xh = big.tile([128, J * OD], F16)          # resident x_hat, fp16
            xall = small.tile([128, J * B], F16, tag="xall")
            dl = small.tile([128, B], F16, tag="dl")
            nc.sync.dma_start(xall[:], xt_d)
            nc.sync.dma_start(dl[:], dl_d)

            sg = small.tile([128, OD], F16, tag="sg")   # all-reduced s, rows 0:64 == 64:128
            ssb16 = small.tile([B, OD], F16, tag="ssb16")  # AR + output staging
            blog = small.tile([128, J * O], F32, tag="blog")  # accumulated logits
            qh = small.tile([128, J * O], F16, tag="qh")      # chunk-b partial dots

            for rep in range(repeat):
                ar_of = {}
                for h in range(2):
                    for it in range(2):
                        w = OD if (ar_chunks == 1 and h == 0) else HALF
                        ai = dram.tile([B, w], F16, tag=f"ar_in{it}{h}_{rep}",
                                       name=f"ar_in{it}{h}_{rep}")
                        ao = dram.tile([B, w], F16, tag=f"ar_out{it}{h}_{rep}",
                                       name=f"ar_out{it}{h}_{rep}", addr_space="Shared")
                        ar_of[(it, h)] = (ai, ao)

                sp = spool.tile([B, OD], F32, tag="sp")

                # ---- production: x_hat tiles + fp16 eviction + iter0 dsum ----
                # dsum0 matmuls are emitted LAG j's behind production so the
                # in-order PE queue never waits on a fresh eviction.
                LAG = 4

                def dsum0_j(j, gs):
                    for g in gs:
                        nc.tensor.matmul(
                            sp[:, g * 512:(g + 1) * 512],
                            dl[:],
                            xh[:, j * OD + g * 512:j * OD + (g + 1) * 512],
                            start=(j == 0), stop=(j == J - 1),
                        )

                ev = 0
                for j in range(J):
                    w_j = wp.tile([128, OD], F16, tag="w")
                    nc.sync.dma_start(w_j[:], wt_d[j])
                    if True:
                        for half in range(2):
                            for g in range(2):
                                pt = ppool.tile([128, 512], F32, tag="pt")
                                od0 = half * 1024 + g * 512
                                for i2 in range(2):
                                    nc.tensor.matmul(
                                        pt[i2 * 64:(i2 + 1) * 64, :],
                                        xall[i2 * 64:(i2 + 1) * 64,
                                             j * B:(j + 1) * B],
                                        w_j[i2 * 64:(i2 + 1) * 64,
                                            od0:od0 + 512],
                                        start=True, stop=True,
                                        tile_position=(i2 * 64, i2 * 64),
                                    )
                                dst = xh[:, j * OD + half * 1024 + g * 512:
                                         j * OD + half * 1024 + g * 512 + 512]
                                if ev % 2 == 0:
                                    nc.vector.tensor_copy(dst, pt[:])
                                else:
                                    nc.scalar.copy(dst, pt[:])
                                ev += 1
                        if j >= LAG:
                            dsum0_j(j - LAG, (0, 1, 2, 3))
                for j in range(J - LAG, J):
                    dsum0_j(j, (0, 1, 2, 3))

                if stage == 1:
                    nc.vector.tensor_copy(ssb16[:], sp[:])
                    nc.sync.dma_start(s2_d, ssb16[:])
                    continue

                # ---- AllReduce d-chunk(s): psum -> sbuf(fp16) -> dram -> CC -> sg
                def allreduce_chunk(it, h, scale, width):
                    c0 = h * HALF
                    if scale == 1.0:
                        nc.scalar.copy(ssb16[:, c0:c0 + width], sp[:, c0:c0 + width])
                    else:
                        nc.scalar.mul(ssb16[:, c0:c0 + width], sp[:, c0:c0 + width],
                                      scale)
                    ai, ao = ar_of[(it, h)]
                    if skip_ar:
                        nc.sync.dma_start(sg[0:64, c0:c0 + width],
                                          ssb16[:, c0:c0 + width])
                        nc.sync.dma_start(sg[64:128, c0:c0 + width],
                                          ssb16[:, c0:c0 + width])
                        return
                    nc.sync.dma_start(ai[:], ssb16[:, c0:c0 + width])
                    nc.gpsimd.collective_compute(
                        "AllReduce", ALU.add,
                        replica_groups=[list(range(CORES))],
                        ins=[ai.opt()], outs=[ao.opt()],
                    )
                    nc.sync.dma_start(sg[0:64, c0:c0 + width], ao[:])
                    nc.sync.dma_start(sg[64:128, c0:c0 + width], ao[:])

                def allreduce_s(it, scale):
                    if ar_chunks == 2:
                        allreduce_chunk(it, 0, scale, HALF)
                        allreduce_chunk(it, 1, scale, HALF)
                    else:
                        allreduce_chunk(it, 0, scale, OD)

                # ---- n2 partial for one d-half: n2 += sum_d sg^2 ------------
                def n2_half(h, n2):
                    c0 = h * HALF
                    sq = scrp.tile([128, HALF], F16, tag="sq", bufs=1)
                    nc.vector.tensor_mul(sq[:], sg[:, c0:c0 + HALF],
                                         sg[:, c0:c0 + HALF])
                    nt = stats.tile([128, O], F32, tag=f"n2h{h}")
                    nc.vector.reduce_sum(nt[:],
                                         sq.rearrange("p (d o) -> p o d", o=O),
                                         axis=AX)
                    if h == 0:
                        nc.vector.tensor_copy(n2[:], nt[:])
                    else:
                        nc.vector.tensor_add(n2[:], n2[:], nt[:])

                # ---- squash factor f = n2 / ((1+n2)(n+eps)), n = sqrt(n2) ---
                def squash_factor(n2, ff):
                    n1 = stats.tile([128, O], F32, tag="n1")
                    # sqrt via exp(0.5*ln(.)): keeps ACT on one table set
                    nc.scalar.activation(n1[:], n2[:], AF.Ln)
                    nc.scalar.activation(n1[:], n1[:], AF.Exp, scale=0.5)
                    t1 = stats.tile([128, O], F32, tag="t1")
                    nc.vector.tensor_scalar_add(t1[:], n2[:], 1.0)
                    nc.vector.reciprocal(t1[:], t1[:])
                    t2 = stats.tile([128, O], F32, tag="t2")
                    nc.vector.tensor_scalar_add(t2[:], n1[:], EPS)
                    nc.vector.reciprocal(t2[:], t2[:])
                    nc.vector.tensor_mul(ff[:], n2[:], t1[:])
                    nc.vector.tensor_mul(ff[:], ff[:], t2[:])

                # ---- dot products q_h[b,(j,o)] = sum_{d in half h} xh*sg ----
                # chunk a (h=0) lands in qa (fp32); chunk b (h=1) folds into
                # qh and is immediately combined into the logits per-jp so the
                # small tail ops pipeline under the next jp's folds:
                #   it==1: blog  = f*(qa+qb);  it==2: blog += f*(qa+qb)
                def dots_half(h, it, ff):
                    c0 = h * HALF
                    DQ = 8   # widest that fits SBUF; folds consume t on the
                    # same in-order DVE queue so one buffer suffices
                    for jp in range(J // DQ):
                        j0 = DQ * jp
                        t = tmpp.tile([128, DQ * HALF], F16, tag="td", bufs=1)
                        t3 = t.rearrange("p (j q) -> p j q", q=HALF)
                        nc.vector.tensor_tensor(
                            t3,
                            xh.rearrange("p (j q) -> p j q", q=OD)
                            [:, j0:j0 + DQ, c0:c0 + HALF],
                            sg[:, c0:c0 + HALF].unsqueeze(1)
                            .broadcast_to([128, DQ, HALF]),
                            ALU.mult,
                        )
                        # fold d 32 -> 1 (d-major layout: halves are contiguous)
                        nc.vector.tensor_add(t3[:, :, 0:512], t3[:, :, 0:512],
                                             t3[:, :, 512:1024])
                        nc.vector.tensor_add(t3[:, :, 0:256], t3[:, :, 0:256],
                                             t3[:, :, 256:512])
                        nc.vector.tensor_add(t3[:, :, 0:128], t3[:, :, 0:128],
                                             t3[:, :, 128:256])
                        nc.vector.tensor_add(t3[:, :, 0:64], t3[:, :, 0:64],
                                             t3[:, :, 64:128])
                        cols = slice(j0 * O, (j0 + DQ) * O)
                        dst = qa if h == 0 else qh
                        nc.vector.tensor_add(
                            dst[:, cols].rearrange("p (j o) -> p j o", o=O),
                            t3[:, :, 0:32], t3[:, :, 32:64])
                    if h == 1:
                        # combine once at full width: fewer DVE ops beats
                        # per-block tails (all of this is serial on the DVE
                        # queue either way)
                        nc.vector.tensor_add(qa[:], qa[:], qh[:])
                        q3 = qa.rearrange("p (j o) -> p j o", o=O)
                        fb = ff.unsqueeze(1).broadcast_to([128, J, O])
                        if it == 1:
                            nc.vector.tensor_tensor(
                                blog.rearrange("p (j o) -> p j o", o=O),
                                q3, fb, ALU.mult)
                        else:
                            nc.vector.tensor_tensor(q3, q3, fb, ALU.mult)
                            nc.vector.tensor_add(blog[:], blog[:], qa[:])

                # ---- softmax over o (innermost of (j,o)) -> cc fp16 ----------
                def softmax(cc):
                    # no max-subtraction: logits are bounded (|b| < ~20); exp
                    # is exact enough in fp32
                    ee = scrp.tile([128, J * O], F32, tag="ee", bufs=1)
                    ee3 = ee.rearrange("p (j o) -> p j o", o=O)
                    nc.scalar.activation(ee[:], blog[:], AF.Exp)
                    den = stats.tile([128, J], F32, tag="den")
                    nc.vector.reduce_sum(den[:], ee3, axis=AX)
                    nc.vector.reciprocal(den[:], den[:])
                    nc.vector.tensor_tensor(
                        cc.rearrange("p (j o) -> p j o", o=O), ee3,
                        den.unsqueeze(2).broadcast_to([128, J, O]), ALU.mult)

                # ---- weighted sum for d-half h: DVE mult + PE dsum ----------
                def weighted_half(cc, h):
                    c0 = h * HALF
                    for jp in range(J // QW):
                        j0 = QW * jp
                        xc = tmpp.tile([128, QW * HALF], F16, tag="tmp4", bufs=2)
                        nc.vector.tensor_tensor(
                            xc.rearrange("p (j d o) -> p j d o", d=D // 2, o=O),
                            xh.rearrange("p (j q) -> p j q", q=OD)
                            [:, j0:j0 + QW, c0:c0 + HALF]
                            .rearrange("p j (d o) -> p j d o", o=O),
                            cc[:, j0 * O:(j0 + QW) * O]
                            .rearrange("p (j o) -> p j o", o=O)
                            .unsqueeze(2).broadcast_to([128, QW, D // 2, O]),
                            ALU.mult,
                        )
                        for q in range(QW):
                            for g in range(2):
                                nc.tensor.matmul(
                                    sp[:, c0 + g * 512:c0 + (g + 1) * 512],
                                    dl[:],
                                    xc[:, q * HALF + g * 512:
                                       q * HALF + (g + 1) * 512],
                                    start=(jp == 0 and q == 0),
                                    stop=(jp == J // QW - 1 and q == QW - 1),
                                )

                # ================= routing =================
                # iteration 0: c uniform -> s0 = (1/32) sum_i x_hat (in sp)
                n2 = stats.tile([128, O], F32, tag="n2")
                ff = stats.tile([128, O], F32, tag="ff")
                qa = scrp.tile([128, J * O], F32, tag="qa", bufs=1)

                for it in (1, 2):
                    sc = (1.0 / O) if it == 1 else 1.0
                    allreduce_s(it - 1, sc)
                    n2_half(0, n2)
                    dots_half(0, it, ff)
                    n2_half(1, n2)
                    squash_factor(n2, ff)
                    dots_half(1, it, ff)
                    if stage == 2 * it:   # logits after iteration `it`
                        nc.vector.tensor_copy(ssb16[:, 0:J * O], blog[0:64, :])
                        nc.sync.dma_start(s2_d, ssb16[:])
                        break
                    cc = scrp.tile([128, J * O], F16, tag="cc", bufs=1)
                    softmax(cc)
                    weighted_half(cc, 0)
                    if it == 2:
                        # stream the output out per d-half as its dsums stop
                        nc.scalar.copy(ssb16[:, 0:HALF], sp[:, 0:HALF])
                        nc.sync.dma_start(s2_d[:, 0:HALF], ssb16[:, 0:HALF])
                    weighted_half(cc, 1)
                    if stage == 2 * it + 1:
                        nc.vector.tensor_copy(ssb16[:], sp[:])
                        nc.sync.dma_start(s2_d, ssb16[:])
                        break
                    if it == 2:
                        nc.scalar.copy(ssb16[:, HALF:OD], sp[:, HALF:OD])
                        nc.sync.dma_start(s2_d[:, HALF:OD], ssb16[:, HALF:OD])

    nc.compile()
    return nc


def _prep(x, weight):
    """Host-side shard + relayout + fp16 cast."""
    x16 = x.astype(np.float16)
    w16 = weight.astype(np.float16)
    xs, ws = [], []
    for c in range(CORES):
        xc = x16[:, c * IL:(c + 1) * IL, :]                 # [B, IL, M]
        xc = xc.reshape(B, J, 2, M).transpose(2, 3, 1, 0)   # [i2, m, j, b]
        xs.append(np.ascontiguousarray(xc.reshape(128, J * B)))
        wc = w16[:, c * IL:(c + 1) * IL, :, :]              # [O, IL, D, M]
        # [o, j, i2, d, m] -> [j, i2, m, d, o]
        wc = wc.reshape(O, J, 2, D, M).transpose(1, 2, 4, 3, 0)
        ws.append(np.ascontiguousarray(wc.reshape(J, 128, OD)))
    dl = np.concatenate([np.eye(B, dtype=np.float16)] * 2, axis=0)  # [128, B]
    return xs, ws, dl


def _squash_np(v):
    n = np.linalg.norm(v, axis=-1, keepdims=True)
    n2 = n * n
    return (n2 / (1.0 + n2)) * v / (n + EPS)


class _Runner:
    """Compile once, execute many times.

    Mirrors the multi-core axon branch of
    concourse.bass_utils.run_bass_kernel_spmd (which lowers through
    bass2jax.run_bass_via_pjrt), but keeps the jitted executable alive so
    repeated calls don't retrace/recompile.
    """

    def __init__(self, nc):
        import jax
        import jax.numpy as jnp  # noqa: F401
        from jax.sharding import Mesh, PartitionSpec
        from jax.experimental.shard_map import shard_map
        from concourse import bass2jax
        from concourse.bass2jax import install_neuronx_cc_hook

        install_neuronx_cc_hook()
        self.nc = nc
        partition_name = (nc.partition_id_tensor.name
                          if nc.partition_id_tensor else None)
        in_names, out_names, out_avals, zero_outs = [], [], [], []
        for alloc in nc.m.functions[0].allocations:
            if not isinstance(alloc, mybir.MemoryLocationSet):
                continue
            name = alloc.memorylocations[0].name
            if alloc.kind == "ExternalInput":
                if name != partition_name:
                    in_names.append(name)
            elif alloc.kind == "ExternalOutput":
                out_names.append(name)
                shape = tuple(alloc.tensor_shape)
                dtype = mybir.dt.np(alloc.dtype)
                out_avals.append(jax.core.ShapedArray(shape, dtype))
                zero_outs.append(np.zeros(shape, dtype))
        n_params = len(in_names)
        n_outs = len(out_avals)
        all_in_names = list(in_names) + list(out_names)
        if partition_name is not None:
            all_in_names.append(partition_name)
        self.in_names = in_names
        self.out_names = out_names
        self.zero_outs = zero_outs
        self.out_avals = out_avals

        def _body(*args):
            operands = list(args)
            if partition_name is not None:
                operands.append(bass2jax.partition_id_tensor())
            outs = bass2jax._bass_exec_p.bind(
                *operands,
                out_avals=tuple(out_avals),
                in_names=tuple(all_in_names),
                out_names=tuple(out_names),
                lowering_input_output_aliases=(),
                sim_require_finite=True,
                sim_require_nnan=True,
                nc=nc,
            )
            return tuple(outs)

        devices = jax.devices()[:CORES]
        assert len(devices) == CORES
        mesh = Mesh(np.asarray(devices), ("core",))
        in_specs = (PartitionSpec("core"),) * (n_params + n_outs)
        out_specs = (PartitionSpec("core"),) * n_outs
        donate = tuple(range(n_params, n_params + n_outs))
        self.sharded = jax.jit(
            shard_map(_body, mesh=mesh, in_specs=in_specs,
                      out_specs=out_specs, check_rep=False),
            donate_argnums=donate, keep_unused=True,
        )

    def __call__(self, in_maps):
        concat_in = [
            np.concatenate([np.asarray(m[name]) for m in in_maps], axis=0)
            for name in self.in_names
        ]
        concat_zeros = [
            np.zeros((CORES * z.shape[0], *z.shape[1:]), z.dtype)
            for z in self.zero_outs
        ]
        out_arrs = self.sharded(*concat_in, *concat_zeros)
        return [
            {
                name: np.asarray(out_arrs[i]).reshape(
                    CORES, *self.out_avals[i].shape)[c]
                for i, name in enumerate(self.out_names)
            }
            for c in range(CORES)
        ]


_RUNNERS = {}


def _get_runner(debug=False, repeat=1, stage=7, skip_ar=False, ar_chunks=2):
    key = (debug, repeat, stage, skip_ar, ar_chunks)
    if key not in _RUNNERS:
        _RUNNERS[key] = _Runner(_build(debug, repeat, stage, skip_ar, ar_chunks))
    return _RUNNERS[key]


def make_in_maps(x, weight):
    xs, ws, dl = _prep(np.asarray(x, np.float32), np.asarray(weight, np.float32))
    return [{"xt": xs[c], "wt": ws[c], "dl": dl} for c in range(CORES)]


def finish(results):
    s2 = np.zeros((B, OD), np.float32)
    for c in range(CORES):
        s2 += results[c]["s2out"]
    s2 = s2.reshape(B, D, O).transpose(0, 2, 1)
    return _squash_np(s2).astype(np.float32)


def kernel(x, weight):
    runner = _get_runner(debug=False)
    results = runner(make_in_maps(x, weight))
    return finish(results)


if __name__ == "__main__":
    rng = np.random.default_rng(0)
    x = rng.standard_normal((B, I, M)).astype(np.float32)
    w = rng.standard_normal((O, I, D, M)).astype(np.float32) * 0.1
    t0 = time.time()
    out = kernel(x, w)
    print("first call (incl compile):", time.time() - t0, "s; out", out.shape)
